# revision 1
# baseline (speedup 1.0000x reference)
"""Trainium2 Bass kernel for nn_CosineSimCausalTransformer (8 NeuronCores).

Sharding: token-parallel over 8 cores. Core c owns 256 tokens:
  chunk 0 = batch 0 rows [128c, 128c+128)
  chunk 1 = batch 1 rows [128(7-c), 128(8-c))
Per layer each core computes QKV for its tokens (all heads), l2-normalizes
q and k locally, AllGathers (kn, v) in bf16 (Shared-output HBM collective),
then runs attention + FFN for its tokens. The final LN output is AllGathered
and the logit projection is vocab-sharded (each core computes all 2048
tokens x V/8 vocab columns).

Activations live transposed in SBUF: [128 part, DT, 256] with d = 128*j + p,
token t = 128*chunk + row. All matmuls keep this feature-major orientation;
only the attention output needs PE transposes back to feature-major.
Causality is enforced via per-core mask data (program is rank-uniform).

Perf notes:
 - All weights are host-prepacked into per-tile SBUF layouts so every DMA
   reads contiguous >=2KB runs per partition (descriptor-efficient).
 - The kn/v AllGather is issued before q-normalize so q work overlaps it.
 - LayerNorm sum-reductions are fused into the preceding out-proj/FFN2
   loops; rsqrt is computed as exp(-0.5*ln(x)) so the ACT engine stays in
   the natural_log_exp table set (shared with attention's Exp) and only
   Gelu forces a table switch.
"""
import numpy as np

import concourse.bacc as bacc
import concourse.mybir as mybir
import concourse.tile as tile
from concourse import bass
from concourse.bass_utils import run_bass_kernel_spmd
from concourse.masks import make_identity

F32 = mybir.dt.float32
BF16 = mybir.dt.bfloat16
I32 = mybir.dt.int32
AF = mybir.ActivationFunctionType
ALU = mybir.AluOpType

NCORES = 8

FULL_CFG = dict(B=2, N=1024, D=1024, H=16, DH=64, V=32000, L=6, F=4096,
                SCALE=8.0, EPS=1e-5)
MINI_CFG = dict(B=2, N=1024, D=256, H=4, DH=64, V=1024, L=2, F=512,
                SCALE=8.0, EPS=1e-5)


def _derive(cfg):
    d = dict(cfg)
    d["DT"] = cfg["D"] // 128
    d["QT"] = cfg["H"] * cfg["DH"] // 128
    d["FT"] = cfg["F"] // 128
    d["HG"] = max(1, cfg["H"] // 4)
    d["GH"] = min(4, cfg["H"])
    d["Vc"] = cfg["V"] // NCORES
    d["T"] = 256
    d["TOK"] = cfg["B"] * cfg["N"]
    assert cfg["N"] == 1024 and cfg["B"] == 2 and cfg["DH"] == 64
    assert cfg["H"] % 2 == 0 and cfg["H"] * cfg["DH"] % 128 == 0
    d["KNE"] = cfg["H"] * cfg["DH"] * d["T"]
    d["VE"] = d["T"] * cfg["H"] * (cfg["DH"] + 1)
    d["KVE"] = d["KNE"] + d["VE"]
    d["HFE"] = cfg["D"] * d["T"]
    return d


def bcast_free(ap, n, axis=1):
    """Insert a step-0 broadcast axis into an AP at free-dim position."""
    pat = list(ap.ap)
    pat.insert(axis, [0, n])
    return bass.AP(tensor=ap.tensor, offset=ap.offset, ap=pat)


def build_kernel(cfg, ablate=()):
    ablate = set(ablate)
    c = _derive(cfg)
    B, N, D, H, DH, V, L, F = (cfg[k] for k in "B N D H DH V L F".split())
    DT, QT, FT, HG, GH, Vc, T, TOK = (c[k] for k in
                                      "DT QT FT HG GH Vc T TOK".split())
    KNE, VE, KVE, HFE = c["KNE"], c["VE"], c["KVE"], c["HFE"]
    SCALE, EPS = cfg["SCALE"], cfg["EPS"]
    HP = H // 2
    MT = (Vc + 127) // 128
    MW = [128] * (MT - 1) + [Vc - 128 * (MT - 1)]
    VW = min(512, H * DH)         # v-projection slab width
    NV = (H * DH) // VW           # v slabs

    nc = bacc.Bacc(num_devices=NCORES)
    Lp = max(L, 1)

    tok_idx = nc.dram_tensor("tok_idx", [128, 2], I32, kind="ExternalInput")
    token_emb = nc.dram_tensor("token_emb", [V, D], F32, kind="ExternalInput")
    pos_embT = nc.dram_tensor("pos_embT", [128, DT, T], BF16, kind="ExternalInput")
    masks = nc.dram_tensor("masks", [128, 2, 8, 128], BF16, kind="ExternalInput")
    w_qkvP = nc.dram_tensor("w_qkvP", [Lp, 2 * QT, 128, DT, 128], BF16,
                            kind="ExternalInput")
    w_vP = nc.dram_tensor("w_vP", [Lp, NV, 128, DT, VW], BF16,
                          kind="ExternalInput")
    w_outP = nc.dram_tensor("w_outP", [Lp, DT, 128, QT, 128], BF16,
                            kind="ExternalInput")
    w1P = nc.dram_tensor("w1P", [Lp, FT, 128, DT, 128], BF16,
                         kind="ExternalInput")
    w2P = nc.dram_tensor("w2P", [Lp, DT, 128, FT, 128], BF16,
                         kind="ExternalInput")
    ln_g = nc.dram_tensor("ln_g", [128, 2 * L + 1, DT], F32, kind="ExternalInput")
    ln_b = nc.dram_tensor("ln_b", [128, 2 * L + 1, DT], F32, kind="ExternalInput")
    b1_in = nc.dram_tensor("b1", [128, Lp, FT], F32, kind="ExternalInput")
    b2_in = nc.dram_tensor("b2", [128, Lp, DT], F32, kind="ExternalInput")
    w_logitP = nc.dram_tensor("w_logitP", [MT, 128, DT, 128], BF16,
                              kind="ExternalInput")
    b_logit = nc.dram_tensor("b_logit", [128, MT], F32, kind="ExternalInput")
    sel2_in = nc.dram_tensor("sel2", [2, 128], F32, kind="ExternalInput")
    out = nc.dram_tensor("logitsT", [Vc, TOK], BF16, kind="ExternalOutput")

    with tile.TileContext(nc) as tc:
        import contextlib
        with contextlib.ExitStack() as ctx:
            sb = ctx.enter_context(tc.tile_pool(name="sb", bufs=1))
            tmp = ctx.enter_context(tc.tile_pool(name="tmp", bufs=3))
            emb = ctx.enter_context(tc.tile_pool(name="emb", bufs=2))
            wpool = ctx.enter_context(tc.tile_pool(name="wp", bufs=3))
            wbig = ctx.enter_context(tc.tile_pool(name="wbig", bufs=2))
            attp = ctx.enter_context(tc.tile_pool(name="attp", bufs=3))
            mmp = ctx.enter_context(tc.tile_pool(name="mmp", bufs=3, space="PSUM"))
            op = ctx.enter_context(tc.tile_pool(name="op", bufs=1, space="PSUM"))
            lnp = ctx.enter_context(tc.tile_pool(name="lnp", bufs=1, space="PSUM"))
            dram = ctx.enter_context(tc.tile_pool(name="dram", bufs=2, space="DRAM"))

            # persistent SBUF
            hT = sb.tile([128, DT, T], F32, tag="hT")
            xnT = sb.tile([128, DT, T], BF16, tag="xnT")
            knL = sb.tile([128, QT, T], BF16, tag="knL")
            vL = sb.tile([128, 2, H, DH + 1], BF16, tag="vL")
            oT = sb.tile([128, QT, T], BF16, tag="oT")
            mask_sb = sb.tile([128, 2, 8, 128], BF16, tag="mask")
            pos_sb = sb.tile([128, DT, T], BF16, tag="pos")
            ident = sb.tile([128, 128], F32, tag="ident")
            ones128 = sb.tile([128, 1], BF16, tag="ones128")
            bd2 = sb.tile([128, 2], BF16, tag="bd2")      # block-diag ones
            sel2 = sb.tile([2, 128], F32, tag="sel2")     # parity selector
            lng_sb = sb.tile([128, 2 * L + 1, DT], F32, tag="lng")
            lnb_sb = sb.tile([128, 2 * L + 1, DT], F32, tag="lnb")
            b1_sb = sb.tile([128, Lp, FT], F32, tag="b1")
            b2_sb = sb.tile([128, Lp, DT], F32, tag="b2")
            blog_sb = sb.tile([128, MT], F32, tag="blog")
            idx_sb = sb.tile([128, 2], I32, tag="idx")
            stats = sb.tile([1, 5, T], F32, tag="stats")
            eps_sb = sb.tile([1, 1], F32, tag="eps")
            rs_sb = sb.tile([128, GH], F32, tag="rs")
            onat = sb.tile([128, 2, H * DH], F32, tag="onat")
            qn_par = sb.tile([128, H, T], BF16, tag="qn_par")

            make_identity(nc, ident[:])
            nc.vector.memset(eps_sb[:], EPS)
            nc.vector.memset(ones128[:], 1.0)
            nc.vector.memset(bd2[:], 0.0)
            nc.vector.memset(bd2[0:64, 0:1], 1.0)
            nc.vector.memset(bd2[64:128, 1:2], 1.0)
            nc.sync.dma_start(sel2[:], sel2_in[:])
            nc.sync.dma_start(pos_sb[:], pos_embT[:])
            nc.sync.dma_start(lng_sb[:], ln_g[:])
            nc.sync.dma_start(lnb_sb[:], ln_b[:])
            nc.sync.dma_start(b1_sb[:], b1_in[:])
            nc.sync.dma_start(b2_sb[:], b2_in[:])
            nc.sync.dma_start(blog_sb[:], b_logit[:])
            nc.sync.dma_start(idx_sb[:], tok_idx[:])
            nc.sync.dma_start(mask_sb[:], masks[:])

            nc.vector.memset(qn_par[:], 0.0)

            def ln_sums(dj, s01):
                """Accumulate [sum(h), sum(h^2)] over feature tiles in one
                PSUM group (single bank)."""
                hb2 = tmp.tile([128, 2, T], BF16, tag="hb2")
                nc.scalar.copy(hb2[:, 0, :], hT[:, dj, :])
                nc.scalar.square(hb2[:, 1, :], hT[:, dj, :])
                nc.tensor.matmul(s01[:], ones128[:], hb2[:],
                                 start=(dj == 0), stop=(dj == DT - 1))

            def ln_finish(gi, s01):
                """stats from s01=[sum, sumsq]; normalize hT -> xnT, row gi."""
                # rows: 0 mean, 1 meansq, 2 var, 3 rstd, 4 -mean*rstd
                nc.vector.tensor_scalar_mul(stats[:, 0, :], s01[:, 0, :], 1.0 / D)
                nc.vector.tensor_scalar_mul(stats[:, 1, :], s01[:, 1, :], 1.0 / D)
                nc.vector.tensor_tensor(out=stats[:, 2, :], in0=stats[:, 0, :],
                                        in1=stats[:, 0, :], op=ALU.mult)
                nc.vector.tensor_tensor(out=stats[:, 2, :], in0=stats[:, 1, :],
                                        in1=stats[:, 2, :], op=ALU.subtract)
                nc.scalar.activation(stats[:, 3, :], stats[:, 2, :], AF.Ln,
                                     bias=eps_sb[:])
                nc.scalar.activation(stats[:, 3, :], stats[:, 3, :], AF.Exp,
                                     scale=-0.5)
                nc.vector.scalar_tensor_tensor(
                    out=stats[:, 4, :], in0=stats[:, 0, :], scalar=-1.0,
                    op0=ALU.mult, op1=ALU.mult, in1=stats[:, 3, :])
                rstd_b = tmp.tile([128, T], F32, tag="rstd_b")
                mb_b = tmp.tile([128, T], F32, tag="mb_b")
                nc.gpsimd.partition_broadcast(rstd_b[:], stats[:, 3, :])
                nc.gpsimd.partition_broadcast(mb_b[:], stats[:, 4, :])
                for dj in range(DT):
                    t1 = tmp.tile([128, T], F32, tag="t1")
                    nc.vector.tensor_tensor(out=t1[:], in0=hT[:, dj, :],
                                            in1=rstd_b[:], op=ALU.mult)
                    nc.vector.tensor_tensor(out=t1[:], in0=t1[:], in1=mb_b[:],
                                            op=ALU.add)
                    nc.vector.tensor_scalar(
                        out=xnT[:, dj, :], in0=t1[:],
                        scalar1=lng_sb[:, gi, dj:dj + 1],
                        scalar2=lnb_sb[:, gi, dj:dj + 1],
                        op0=ALU.mult, op1=ALU.add)

            def qk_normalize(li, dst, m0, parity_split):
                """l2norm of (w_qkv cols [m0+m]) @ xn; head pair per m-tile."""
                for m in range(QT):
                    wc = wpool.tile([128, DT, 128], BF16, tag="wcol")
                    nc.sync.dma_start(wc[:], w_qkvP[li, m0 + m])
                    ps = mmp.tile([128, T], F32, tag="mm")
                    for dj in range(DT):
                        nc.tensor.matmul(ps[:], wc[:, dj, :], xnT[:, dj, :],
                                         start=(dj == 0), stop=(dj == DT - 1))
                    raw = tmp.tile([128, T], BF16, tag="qraw")
                    sqq = tmp.tile([128, T], BF16, tag="qsq")
                    nc.scalar.copy(raw[:], ps[:])
                    nc.scalar.square(sqq[:], ps[:])
                    # per-head-sums via block-diag, K=128
                    ss = mmp.tile([2, T], F32, tag="mm")
                    nc.tensor.matmul(ss[:], bd2[:], sqq[:], start=True, stop=True)
                    rql = tmp.tile([2, T], F32, tag="rql")
                    nc.scalar.activation(rql[:], ss[:], AF.Ln)
                    nc.scalar.activation(rql[:], rql[:], AF.Exp, scale=-0.5)
                    # parity-select broadcast back to 128 partitions, K=2
                    rq_ps = mmp.tile([128, T], F32, tag="mm")
                    nc.tensor.matmul(rq_ps[:], sel2[:], rql[:], start=True,
                                     stop=True)
                    if parity_split:
                        for hh in range(2):
                            pb = 64 * hh
                            nc.vector.tensor_tensor(
                                out=dst[pb:pb + 64, 2 * m + hh, :],
                                in0=raw[pb:pb + 64, :],
                                in1=rq_ps[pb:pb + 64, :], op=ALU.mult)
                    else:
                        nc.vector.tensor_tensor(out=dst[:, m, :], in0=raw[:],
                                                in1=rq_ps[:], op=ALU.mult)

            # ---- embedding gather + transpose + positional add + LN0 sums ----
            h0s = []
            for k in range(2):
                h0 = emb.tile([128, D], F32, tag="h0")
                nc.gpsimd.indirect_dma_start(
                    out=h0[:], out_offset=None, in_=token_emb[:],
                    in_offset=bass.IndirectOffsetOnAxis(ap=idx_sb[:, k:k + 1], axis=0),
                )
                h0s.append(h0)
            s01 = lnp.tile([1, 2, T], F32, tag="s01")
            for dj in range(DT):
                for k in range(2):
                    tp = mmp.tile([128, 128], F32, tag="mm")
                    nc.tensor.transpose(tp[:], h0s[k][:, 128 * dj:128 * (dj + 1)],
                                        ident[:])
                    nc.vector.scalar_tensor_tensor(
                        out=hT[:, dj, 128 * k:128 * (k + 1)], in0=tp[:],
                        scalar=1.0, op0=ALU.mult, op1=ALU.add,
                        in1=pos_sb[:, dj, 128 * k:128 * (k + 1)])
                ln_sums(dj, s01)

            for li in range(L):
                # ---------------- attention ----------------
                ln_finish(2 * li, s01)
                qk_normalize(li, knL, QT, False)
                # ---- AllGather kn first; v and q overlap it ----
                kvlk = dram.tile([KNE], BF16, tag="kvlk")
                kvgk = dram.tile([4, KNE], BF16, tag="kvgk")
                nc.sync.dma_start(
                    kvlk.rearrange("(m p t) -> p m t", p=128, t=T), knL[:])
                if "noag" not in ablate:
                    nc.gpsimd.collective_compute(
                        "AllGather", ALU.bypass,
                        replica_groups=[[0, 1, 2, 3], [4, 5, 6, 7]],
                        ins=[kvlk.opt()], outs=[kvgk.opt()])
                nc.vector.memset(vL[:], 1.0)
                for vf in range(NV):
                    wv = wbig.tile([128, DT, VW], BF16, tag="wbig")
                    nc.sync.dma_start(wv[:], w_vP[li, vf])
                    for k in range(2):
                        ps = mmp.tile([128, VW], F32, tag="mm")
                        for dj in range(DT):
                            nc.tensor.matmul(ps[:], xnT[:, dj, 128 * k:128 * (k + 1)],
                                             wv[:, dj, :],
                                             start=(dj == 0), stop=(dj == DT - 1))
                        nh = VW // DH
                        nc.scalar.copy(
                            vL[:, k, nh * vf:nh * (vf + 1), 0:DH],
                            ps[:].rearrange("p (h x) -> p h x", x=DH))
                # ---- AllGather v; q-normalize + sims overlap it ----
                kvlv = dram.tile([VE], BF16, tag="kvlv")
                kvgv = dram.tile([4, VE], BF16, tag="kvgv")
                nc.sync.dma_start(
                    kvlv.rearrange("(k p f) -> p k f", p=128, k=2),
                    vL.rearrange("p k h x -> p k (h x)"))
                if "noag" not in ablate:
                    nc.gpsimd.collective_compute(
                        "AllGather", ALU.bypass,
                        replica_groups=[[0, 1, 2, 3], [4, 5, 6, 7]],
                        ins=[kvlv.opt()], outs=[kvgv.opt()])
                qk_normalize(li, qn_par, 0, True)
                kn_sb = sb.tile([128, HP, 4, 2, 128], BF16, tag="kn_sb")
                v_sb = sb.tile([128, 4, 2, H, DH + 1], BF16, tag="vbig")
                for hp in range(HP):
                    nc.sync.dma_start(
                        kn_sb[:, hp, :, :, :],
                        kvgk[:, 128 * hp * T:128 * (hp + 1) * T]
                        .rearrange("r (p k f) -> p r k f", p=128, k=2))
                for r in range(4):
                    nc.sync.dma_start(
                        v_sb[:, r, :, :, :],
                        kvgv[r].rearrange("(k p h x) -> p k h x",
                                          k=2, p=128, h=H))
                # ---- attention; sims/exps of group i+1 overlap AVs of
                # group i, and the sim phase only depends on the kn AG ----
                groups = [(k, g) for k in range(2) for g in range(HG)]

                def attn_sims(k, g):
                    a_big = attp.tile([128, 8, GH, 128], BF16, tag="a")
                    for j in range(8):
                        r, ks = (j, 0) if j < 4 else (7 - j, 1)
                        s_ps = mmp.tile([128, GH, 128], F32, tag="mm")
                        for hh in range(GH):
                            h = g * GH + hh
                            nc.tensor.matmul(
                                s_ps[:, hh, :],
                                kn_sb[:, h // 2, r, ks, :],
                                qn_par[:, h, 128 * k:128 * (k + 1)],
                                start=True, stop=True)
                        nc.scalar.activation(a_big[:, j], s_ps[:], AF.Exp,
                                             scale=SCALE)
                        nc.vector.tensor_tensor(
                            out=a_big[:, j], in0=a_big[:, j],
                            in1=bcast_free(mask_sb[:, k, j, :], GH),
                            op=ALU.mult)
                    return a_big

                def attn_avs(k, g, a_big):
                    o_ps = op.tile([128, GH, 512], F32, tag="o")
                    for j in range(8):
                        r, ks = (j, 0) if j < 4 else (7 - j, 1)
                        for hh in range(GH):
                            h = g * GH + hh
                            nc.tensor.matmul(
                                o_ps[:, hh, 0:DH + 1], a_big[:, j, hh, :],
                                v_sb[:, r, ks, h, :],
                                start=(j == 0), stop=(j == 7))
                    nc.vector.reciprocal(rs_sb[:, 0:GH], o_ps[:, :, DH])
                    for hh in range(GH):
                        h = g * GH + hh
                        nc.vector.tensor_scalar_mul(
                            onat[:, k, DH * h:DH * (h + 1)],
                            o_ps[:, hh, 0:DH], rs_sb[:, hh:hh + 1])

                prev = None
                for (k, g) in groups:
                    a_big = attn_sims(k, g)
                    if prev is not None:
                        attn_avs(*prev)
                    prev = (k, g, a_big)
                attn_avs(*prev)
                for k in range(2):
                    for mj in range(QT):
                        tp = mmp.tile([128, 128], F32, tag="mm")
                        nc.tensor.transpose(tp[:], onat[:, k, 128 * mj:128 * (mj + 1)],
                                            ident[:])
                        nc.scalar.copy(oT[:, mj, 128 * k:128 * (k + 1)], tp[:])
                # ---- out-proj + residual, fused LN2 sums ----
                s01 = lnp.tile([1, 2, T], F32, tag="s01")
                for dj in range(DT):
                    wc = wpool.tile([128, QT, 128], BF16, tag="wcol")
                    nc.sync.dma_start(wc[:], w_outP[li, dj])
                    ps = mmp.tile([128, T], F32, tag="mm")
                    for oj in range(QT):
                        nc.tensor.matmul(ps[:], wc[:, oj, :], oT[:, oj, :],
                                         start=(oj == 0), stop=(oj == QT - 1))
                    nc.vector.tensor_tensor(out=hT[:, dj, :], in0=ps[:],
                                            in1=hT[:, dj, :], op=ALU.add)
                    ln_sums(dj, s01)
                # ---------------- FFN ----------------
                ln_finish(2 * li + 1, s01)
                gelu_sb = sb.tile([128, FT, T], BF16, tag="vbig")
                for m in range(FT):
                    wc = wpool.tile([128, DT, 128], BF16, tag="wcol")
                    nc.sync.dma_start(wc[:], w1P[li, m])
                    ps = mmp.tile([128, T], F32, tag="mm")
                    for dj in range(DT):
                        nc.tensor.matmul(ps[:], wc[:, dj, :], xnT[:, dj, :],
                                         start=(dj == 0), stop=(dj == DT - 1))
                    nc.scalar.activation(gelu_sb[:, m, :], ps[:], AF.Gelu,
                                         bias=b1_sb[:, li, m:m + 1], scale=1.0)
                s01 = lnp.tile([1, 2, T], F32, tag="s01")
                for dj in range(DT):
                    w2c = wbig.tile([128, FT, 128], BF16, tag="wbig")
                    nc.sync.dma_start(w2c[:], w2P[li, dj])
                    ps = mmp.tile([128, T], F32, tag="mm")
                    for fj in range(FT):
                        nc.tensor.matmul(ps[:], w2c[:, fj, :], gelu_sb[:, fj, :],
                                         start=(fj == 0), stop=(fj == FT - 1))
                    nc.vector.scalar_tensor_tensor(
                        out=hT[:, dj, :], in0=ps[:],
                        scalar=b2_sb[:, li, dj:dj + 1],
                        op0=ALU.add, op1=ALU.add, in1=hT[:, dj, :])
                    ln_sums(dj, s01)

            # ---------------- final LN + logits (vocab-sharded) ----------------
            ln_finish(2 * L, s01)
            hfl = dram.tile([HFE], BF16, tag="hfl")
            hfg = dram.tile([NCORES, HFE], BF16, tag="hfg", addr_space="Shared")
            nc.sync.dma_start(
                hfl.rearrange("(p m t) -> p m t", p=128, t=T), xnT[:])
            if "noag" not in ablate:
                nc.gpsimd.collective_compute(
                    "AllGather", ALU.bypass, replica_groups=[list(range(NCORES))],
                    ins=[hfl.opt()], outs=[hfg.opt()])
            hf_sb = sb.tile([128, DT, 8, 256], BF16, tag="kn_sb")
            for dj in range(DT):
                nc.sync.dma_start(
                    hf_sb[:, dj, :, :],
                    hfg.rearrange("r (p m t) -> p m r t", p=128, t=T)[:, dj, :, :])
            for m in range(MT):
                mw = MW[m]
                wc = wpool.tile([128, DT, 128], BF16, tag="wcol")
                nc.sync.dma_start(wc[:], w_logitP[m])
                for ts in range(4):
                    ps = mmp.tile([128, 512], F32, tag="mm")
                    for dj in range(DT):
                        nc.tensor.matmul(
                            ps[0:mw, :], wc[:, dj, 0:mw],
                            hf_sb[:, dj, 2 * ts:2 * ts + 2, :],
                            start=(dj == 0), stop=(dj == DT - 1))
                    lsb = tmp.tile([128, 512], BF16, tag="lsb")
                    nc.vector.tensor_scalar(
                        out=lsb[0:mw, :], in0=ps[0:mw, :],
                        scalar1=blog_sb[0:mw, m:m + 1], scalar2=None,
                        op0=ALU.add)
                    nc.sync.dma_start(
                        out[128 * m:128 * m + mw, 512 * ts:512 * (ts + 1)],
                        lsb[0:mw, :])
    _insert_act_loads_smart(nc)
    nc.finalize()
    return nc


def _insert_act_loads_smart(nc):
    """Insert InstLoadActFuncSet with lookahead so consecutive activations
    share one table set (the stock pass greedily picks the first matching
    set and thrashes between Ln/Exp tables). Runs on the scheduled
    instruction order; the stock pass then sees every activation covered."""
    from concourse.hw_specs import get_activation_tables
    tabs = list(get_activation_tables(nc.m.arch).values())
    for block in nc.m.functions[0].blocks:
        insts = list(block.instructions)
        acts = [(i, ins.func) for i, ins in enumerate(insts)
                if isinstance(ins, mybir.InstActivation)]
        cur = None
        need = {}
        for idx, (i, f) in enumerate(acts):
            if cur is not None and f in tabs[cur]:
                continue
            best, bestlen = None, -1
            for si, s in enumerate(tabs):
                if f not in s:
                    continue
                run = 0
                for _, f2 in acts[idx + 1:idx + 64]:
                    if f2 in s:
                        run += 1
                    else:
                        break
                if run > bestlen:
                    best, bestlen = si, run
            assert best is not None, f"no act table serves {f}"
            cur = best
            need[i] = best
        if not need:
            continue
        out = []
        for i, ins in enumerate(insts):
            if i in need:
                ld = mybir.InstLoadActFuncSet(
                    name=nc.get_next_instruction_name(),
                    act_func_set_id=need[i], ins=[], outs=[])
                ld.engine = mybir.EngineType.Activation
                out.append(ld)
            out.append(ins)
        block.instructions[:] = out


# ===================== host side =====================

def _pack2(w, KT, MTT):
    """[K, M] row-major -> [MTT, 128, KT, 128] per-tile SBUF layout."""
    return np.ascontiguousarray(
        w.reshape(KT, 128, MTT, 128).transpose(2, 1, 0, 3))


def prepare_inputs(cfg, inputs):
    import ml_dtypes
    BF = ml_dtypes.bfloat16
    c = _derive(cfg)
    B, N, D, H, DH, V, L, F = (cfg[k] for k in "B N D H DH V L F".split())
    DT, QT, FT, Vc, T = c["DT"], c["QT"], c["FT"], c["Vc"], c["T"]
    MT = (Vc + 127) // 128
    VW = min(512, H * DH)
    NV = (H * DH) // VW
    Lp = max(L, 1)

    def to_bf(a):
        return np.asarray(a, dtype=np.float32).astype(BF)

    x = np.asarray(inputs["x"]).astype(np.int32)
    token_emb = np.ascontiguousarray(np.asarray(inputs["token_emb"], np.float32))
    pos_emb = np.asarray(inputs["pos_emb"], np.float32)

    w_qkv = np.asarray(inputs["w_qkv"], np.float32)
    w_qkvP = np.stack([_pack2(to_bf(w_qkv[l][:, :2 * H * DH]), DT, 2 * QT)
                       for l in range(Lp)])
    w_vP = np.stack([
        np.ascontiguousarray(
            to_bf(w_qkv[l][:, 2 * H * DH:])
            .reshape(DT, 128, NV, VW).transpose(2, 1, 0, 3))
        for l in range(Lp)])
    w_outP = np.stack([_pack2(to_bf(np.asarray(inputs["w_out"][l], np.float32)),
                              QT, DT) for l in range(Lp)])
    w1P = np.stack([_pack2(to_bf(np.asarray(inputs["w1"][l], np.float32)),
                           DT, FT) for l in range(Lp)])
    w2P = np.stack([_pack2(to_bf(np.asarray(inputs["w2"][l], np.float32)),
                           FT, DT) for l in range(Lp)])

    gs, bs = [], []
    for l in range(L):
        gs += [np.asarray(inputs["ln1_g"][l]), np.asarray(inputs["ln2_g"][l])]
        bs += [np.asarray(inputs["ln1_b"][l]), np.asarray(inputs["ln2_b"][l])]
    gs.append(np.asarray(inputs["lnf_g"]))
    bs.append(np.asarray(inputs["lnf_b"]))
    ln_g = np.stack(gs).astype(np.float32).reshape(2 * L + 1, DT, 128)\
        .transpose(2, 0, 1)
    ln_b = np.stack(bs).astype(np.float32).reshape(2 * L + 1, DT, 128)\
        .transpose(2, 0, 1)
    b1 = np.asarray(inputs["b1"], np.float32).reshape(Lp, FT, 128)\
        .transpose(2, 0, 1)
    b2 = np.asarray(inputs["b2"], np.float32).reshape(Lp, DT, 128)\
        .transpose(2, 0, 1)
    w_logit = np.asarray(inputs["w_logit"], np.float32)
    b_logit = np.asarray(inputs["b_logit"], np.float32)

    tri = np.triu(np.ones((128, 128), np.float32)).astype(BF)  # keep key<=row

    in_maps = []
    for core in range(NCORES):
        bb, cc = core // 4, core % 4
        p0, p1 = cc, 7 - cc
        ids = np.stack([x[bb, 128 * p0:128 * (p0 + 1)],
                        x[bb, 128 * p1:128 * (p1 + 1)]])
        posT = np.concatenate([pos_emb[128 * p0:128 * (p0 + 1)],
                               pos_emb[128 * p1:128 * (p1 + 1)]])  # [256, D]
        posT = posT.T.reshape(DT, 128, T).transpose(1, 0, 2).astype(BF)
        mk = np.zeros((2, 8, 128, 128), BF)
        for k, p in ((0, p0), (1, p1)):
            for j in range(8):
                if j < p:
                    mk[k, j] = 1.0
                elif j == p:
                    mk[k, j] = tri
        vs = slice(Vc * core, Vc * (core + 1))
        wl = np.zeros((D, MT * 128), np.float32)
        wl[:, :Vc] = w_logit[:, vs]
        bl = np.zeros((MT * 128,), np.float32)
        bl[:Vc] = b_logit[vs]
        sel2 = np.zeros((2, 128), np.float32)
        sel2[0, 0:64] = 1.0
        sel2[1, 64:128] = 1.0
        in_maps.append({
            "sel2": sel2,
            "tok_idx": np.ascontiguousarray(ids.T),
            "token_emb": token_emb,
            "pos_embT": np.ascontiguousarray(posT),
            "masks": np.ascontiguousarray(mk.transpose(2, 0, 1, 3)),
            "w_qkvP": w_qkvP, "w_vP": w_vP, "w_outP": w_outP,
            "w1P": w1P, "w2P": w2P,
            "ln_g": np.ascontiguousarray(ln_g),
            "ln_b": np.ascontiguousarray(ln_b),
            "b1": np.ascontiguousarray(b1), "b2": np.ascontiguousarray(b2),
            "w_logitP": _pack2(wl.astype(BF), DT, MT),
            "b_logit": np.ascontiguousarray(bl.reshape(MT, 128).T),
        })
    return in_maps


def assemble_output(cfg, results):
    c = _derive(cfg)
    B, N, V = cfg["B"], cfg["N"], cfg["V"]
    Vc = c["Vc"]
    out = np.empty((B, N, V), np.float32)
    for core in range(NCORES):
        lt = np.asarray(results[core]["logitsT"], dtype=np.float32)
        vs = slice(Vc * core, Vc * (core + 1))
        for r in range(8):
            rb, rc = r // 4, r % 4
            out[rb, 128 * rc:128 * (rc + 1), vs] = \
                lt[:, 256 * r:256 * r + 128].T
            out[rb, 128 * (7 - rc):128 * (8 - rc), vs] = \
                lt[:, 256 * r + 128:256 * r + 256].T
    return out


_BUILT = {}


def kernel(**inputs) -> np.ndarray:
    if "full" not in _BUILT:
        _BUILT["full"] = build_kernel(FULL_CFG)
    in_maps = prepare_inputs(FULL_CFG, inputs)
    res = run_bass_kernel_spmd(_BUILT["full"], in_maps, list(range(NCORES)))
    return assemble_output(FULL_CFG, res.results)

